# revision 4
# baseline (speedup 1.0000x reference)
"""DenseFastGAT forward on 8 Trainium2 NeuronCores (Bass/Tile).

Math (per batch b):
  z  = x @ W.T + bW                                  [N, O]
  ai = z @ wai.T + bai ; aj = z @ waj.T + baj        [N]
  e  = leakyrelu(ai_i + aj_j, 0.2)
  att = softmax_row(where(adj>0, e, -9e15) ++ sink(-1e9))[:, :N]
  out = att @ z

Kernel strategy (v2):
  - ai/aj fold to x @ (W.T @ wai.T) + const: computed on host in f64
    (tiny: 2 x [N,256]@[256,1] per batch), fed as vectors.
  - Sharding: 8 cores = 2 batches x 4 row-slabs of NI=1024 rows each.
    Each core gets the full-batch adjacency TRANSPOSED slab (bf16),
    x rows in [128, jt, 257] layout ([x | 1]), and host-folded
    attention vectors.
  - Re-association: out = (att @ [x|1]) @ [W.T] + bW. The main loop
    accumulates Y = p.T @ x_aug (col 256 = softmax denominator, via the
    ones column) — this subsumes the z GEMM, which on this part ran at
    a degraded PE rate due to its short accumulation groups. A small
    tail (PE transpose of Y-hat, 16 matmuls against W.T, bias add)
    produces the output. Exact: softmax rows sum to 1, so the bias
    passes through the attention average.
  - p field in ONE fused VectorE op per j-tile: softmax rows are
    scale-invariant, so scale row i by exp(-0.2*ai_i):
      p'[j,i] = adj * max(exp(0.8*ai_i)*exp(aj_j), exp(0.2*aj_j))
    = tensor_scalar(g_bc, f1[j], f2[j], mult, max) then one mask mult.
    Split 3:1 DVE:Pool per quad so the PE is never starved.
  - No max-subtraction softmax: all fields positive, denominators
    >= 20*exp(-3); bf16 dynamic range covers exp(27).
"""

import numpy as np
import ml_dtypes

B = 2
N = 4096
IN_F = 256
O = 256
NCORES = 8
SLABS_PER_B = 4
NI = N // SLABS_PER_B        # 1024 rows per core
JT = N // 128                # 32 j-tiles
NQ = JT // 4                 # 8 quads of j-tiles
IC = NI // 128               # 8 output chunks per core
KA = IN_F + 1                # 257 (x augmented with ones column)
ALPHA = 0.2

_CACHE = {}


def _build():
    import concourse.bacc as bacc
    import concourse.mybir as mybir
    import concourse.tile as tile

    dt = mybir.dt
    AF = mybir.ActivationFunctionType
    ALU = mybir.AluOpType

    nc = bacc.Bacc("TRN2", target_bir_lowering=False, debug=False,
                   num_devices=NCORES)

    adjsT = nc.dram_tensor("adjsT", [N, NI], dt.bfloat16, kind="ExternalInput")
    x_aug = nc.dram_tensor("x_aug", [128, JT, KA], dt.bfloat16,
                           kind="ExternalInput")
    wT_d = nc.dram_tensor("wT", [128, 2, O], dt.bfloat16, kind="ExternalInput")
    g_row = nc.dram_tensor("g_row", [1, NI], dt.bfloat16, kind="ExternalInput")
    f1c_d = nc.dram_tensor("f1c", [128, JT], dt.float32, kind="ExternalInput")
    f2c_d = nc.dram_tensor("f2c", [128, JT], dt.float32, kind="ExternalInput")
    bias_d = nc.dram_tensor("bias", [1, O], dt.float32, kind="ExternalInput")
    ident_d = nc.dram_tensor("ident", [128, 128], dt.bfloat16,
                             kind="ExternalInput")
    out = nc.dram_tensor("out", [NI, O], dt.float32, kind="ExternalOutput")

    adjq_view = adjsT.ap().rearrange("(q k p) i -> q p k i", k=4, p=128)
    adjj_view = adjsT.ap().rearrange("(t p) i -> t p i", p=128)

    with tile.TileContext(nc) as tc:
        with tc.tile_pool(name="consts", bufs=1) as consts, \
             tc.tile_pool(name="adjp", bufs=3) as adjp, \
             tc.tile_pool(name="pp", bufs=2) as pp, \
             tc.tile_pool(name="ysbp", bufs=1) as ysbp, \
             tc.tile_pool(name="ytp", bufs=2) as ytp, \
             tc.tile_pool(name="outp", bufs=2) as outp, \
             tc.tile_pool(name="smallp", bufs=2) as smallp:

            # ---- constants into SBUF ----
            # Issue order matters: the Sync queue serializes DMA setups
            # (~0.6us each), so adjacency jt0 must be first there; tiny
            # p-build consts ride the gpsimd queue; tail-only consts (wT,
            # ident, bias) are issued after the main loop's DMAs.
            g_bc = consts.tile([128, NI], dt.bfloat16, tag="g_bc")
            nc.gpsimd.dma_start(out=g_bc[:],
                                in_=g_row.ap().to_broadcast([128, NI]))
            f1c = consts.tile([128, JT], dt.float32, tag="f1c")
            f2c = consts.tile([128, JT], dt.float32, tag="f2c")
            nc.gpsimd.dma_start(out=f1c[:], in_=f1c_d[:])
            nc.gpsimd.dma_start(out=f2c[:], in_=f2c_d[:])
            wT_sb = consts.tile([128, 2, O], dt.bfloat16, tag="wT")
            bias_bc = consts.tile([128, O], dt.float32, tag="bias")
            ident = consts.tile([128, 128], dt.bfloat16, tag="ident")
            x_sb = consts.tile([128, JT, KA], dt.bfloat16, tag="x_sb")

            # ---- main loop: Y[ic] += p'.T @ [x|1] over 32 j-tiles ----
            with tc.tile_pool(name="accp", bufs=1, space="PSUM") as accp:
                accs = [accp.tile([128, KA], dt.float32, tag=f"acc{ic}",
                                  name=f"acc{ic}")
                        for ic in range(IC)]

                for q in range(NQ):
                    adjt = adjp.tile([128, 4, NI], dt.bfloat16, name="adjt")
                    if q == 0:
                        # jt-granular so the first p-build starts ASAP
                        for k in range(4):
                            nc.sync.dma_start(out=adjt[:, k, :],
                                              in_=adjj_view[k])
                    else:
                        nc.sync.dma_start(out=adjt[:], in_=adjq_view[q])
                    p_t = pp.tile([128, 4, NI], dt.bfloat16, name="p_t")
                    for k in range(4):
                        jt = q * 4 + k
                        js = slice(jt, jt + 1)
                        eng = nc.gpsimd if k == 3 else nc.vector
                        eng.tensor_scalar(p_t[:, k, :], g_bc[:],
                                          f1c[:, js], f2c[:, js],
                                          op0=ALU.mult, op1=ALU.max)
                    nc.vector.tensor_tensor(p_t[:, 0:3, :], p_t[:, 0:3, :],
                                            adjt[:, 0:3, :], op=ALU.mult)
                    nc.gpsimd.tensor_tensor(p_t[:, 3, :], p_t[:, 3, :],
                                            adjt[:, 3, :], op=ALU.mult)
                    for k in range(4):
                        jt = q * 4 + k
                        for ic in range(IC):
                            nc.tensor.matmul(
                                accs[ic][:],
                                p_t[:, k, ic * 128:(ic + 1) * 128],
                                x_sb[:, jt, :],
                                start=(jt == 0), stop=(jt == JT - 1))

                # ---- normalize: yhat = Y[:, :256] / d (d = col 256) ----
                r_t = smallp.tile([128, IC], dt.float32, tag="r_t")
                ysb = ysbp.tile([128, IC, O], dt.bfloat16, tag="ysb")
                for ic in range(IC):
                    nc.vector.reciprocal(r_t[:, ic:ic + 1],
                                         accs[ic][:, O:O + 1])
                    nc.scalar.activation(ysb[:, ic, :], accs[ic][:, 0:O],
                                         AF.Copy, scale=r_t[:, ic:ic + 1])

            # ---- tail: out = yhat @ W.T + bW (PE transpose + 16 matmuls) --
            with tc.tile_pool(name="tps", bufs=4, space="PSUM") as tps, \
                 tc.tile_pool(name="g2p", bufs=2, space="PSUM") as g2p:
                for ic in range(IC):
                    yt = ytp.tile([128, 2, 128], dt.bfloat16, name="yt")
                    for fh in range(2):
                        tp = tps.tile([128, 128], dt.bfloat16, name="tp")
                        nc.tensor.transpose(
                            tp[:], ysb[:, ic, fh * 128:(fh + 1) * 128],
                            ident[:])
                        if fh == 0:
                            nc.vector.tensor_copy(yt[:, fh, :], tp[:])
                        else:
                            nc.scalar.copy(yt[:, fh, :], tp[:])
                    G = g2p.tile([128, O], dt.float32, name="G")
                    nc.tensor.matmul(G[:], yt[:, 0, :], wT_sb[:, 0, :],
                                     start=True, stop=False)
                    nc.tensor.matmul(G[:], yt[:, 1, :], wT_sb[:, 1, :],
                                     start=False, stop=True)
                    ot = outp.tile([128, O], dt.float32, name="ot")
                    # gpsimd cannot read PSUM; keep the bias adds on DVE
                    nc.vector.tensor_tensor(ot[:], G[:], bias_bc[:], op=ALU.add)
                    nc.sync.dma_start(out=out[ic * 128:(ic + 1) * 128, :],
                                      in_=ot[:])

    nc.compile()
    return nc


def _get_nc():
    if "nc" not in _CACHE:
        _CACHE["nc"] = _build()
    return _CACHE["nc"]


def kernel(x, adjs, W, bW, wai, bai, waj, baj):
    from concourse import bass_utils

    bf16 = ml_dtypes.bfloat16
    x = np.asarray(x, np.float32)
    adjs = np.asarray(adjs, np.float32)
    W = np.asarray(W, np.float32)
    bW = np.asarray(bW, np.float32)
    wai = np.asarray(wai, np.float32)
    bai = np.asarray(bai, np.float32)
    waj = np.asarray(waj, np.float32)
    baj = np.asarray(baj, np.float32)

    # host-folded attention projections (f64 for accuracy)
    u_i = W.astype(np.float64).T @ wai.astype(np.float64).T        # [256,1]
    c_i = float(bW.astype(np.float64) @ wai[0].astype(np.float64)
                + bai.astype(np.float64)[0])
    u_j = W.astype(np.float64).T @ waj.astype(np.float64).T
    c_j = float(bW.astype(np.float64) @ waj[0].astype(np.float64)
                + baj.astype(np.float64)[0])
    ai = (x.astype(np.float64) @ u_i)[:, :, 0] + c_i               # [B,N] f64
    aj = (x.astype(np.float64) @ u_j)[:, :, 0] + c_j

    # per-batch shared inputs
    wT = np.ascontiguousarray(
        W.T.reshape(2, 128, O).transpose(1, 0, 2)).astype(bf16)
    bias = bW.reshape(1, O).astype(np.float32)
    ident = np.eye(128, dtype=np.float32).astype(bf16)

    x_aug_b, f1_b, f2_b = [], [], []
    for b in range(B):
        xa = np.empty((128, JT, KA), bf16)
        xa[:, :, :IN_F] = x[b].reshape(JT, 128, IN_F).transpose(1, 0, 2)
        xa[:, :, IN_F] = np.float32(1.0)
        x_aug_b.append(xa)
        f1_b.append(np.ascontiguousarray(
            np.exp(aj[b]).reshape(JT, 128).T).astype(np.float32))
        f2_b.append(np.ascontiguousarray(
            np.exp(ALPHA * aj[b]).reshape(JT, 128).T).astype(np.float32))

    in_maps = []
    for c in range(NCORES):
        b, s = divmod(c, SLABS_PER_B)
        i0 = s * NI
        adjsT_slab = np.ascontiguousarray(adjs[b][i0:i0 + NI, :].T).astype(bf16)
        g_slab = np.exp(0.8 * ai[b, i0:i0 + NI]).reshape(1, NI).astype(bf16)
        in_maps.append({
            "adjsT": adjsT_slab,
            "x_aug": x_aug_b[b],
            "wT": wT,
            "g_row": g_slab,
            "f1c": f1_b[b],
            "f2c": f2_b[b],
            "bias": bias,
            "ident": ident,
        })

    nc = _get_nc()
    res = bass_utils.run_bass_kernel_spmd(
        nc, in_maps, core_ids=list(range(NCORES)),
        **_CACHE.get("run_kwargs", {}))
    _CACHE["last_results"] = res

    out = np.empty((B, N, O), np.float32)
    for c in range(NCORES):
        b, s = divmod(c, SLABS_PER_B)
        out[b, s * NI:(s + 1) * NI, :] = res.results[c]["out"]
    return out


# revision 7
# speedup vs baseline: 2.3784x; 2.3784x over previous
"""DenseFastGAT forward on 8 Trainium2 NeuronCores (Bass/Tile).

Math (per batch b):
  z  = x @ W.T + bW                                  [N, O]
  ai = z @ wai.T + bai ; aj = z @ waj.T + baj        [N]
  e  = leakyrelu(ai_i + aj_j, 0.2)
  att = softmax_row(where(adj>0, e, -9e15) ++ sink(-1e9))[:, :N]
  out = att @ z

Kernel strategy (v3):
  - ai/aj fold to x @ (W.T @ wai.T) + const: computed on host in f64
    (tiny: 2 x [N,256]@[256,1] per batch), fed as vectors.
  - Sharding: 8 cores = 2 batches x 4 row-slabs of NI=1024 rows each.
  - Re-association: out = (att @ [x|1]) @ W.T + bW. The main loop
    accumulates Y = p.T @ [x|1] (col 256 = softmax denominator via the
    ones column); a small tail (PE transpose of yhat = Y/d, 16 matmuls
    against W.T) projects through W. Exact: softmax rows sum to 1, so
    bW passes through the attention average (host adds it on unshard).
    This removes the separate z GEMM, which ran at a degraded PE rate
    (short accumulation groups never leave the low PE p-state).
  - p field, 2 passes/tile: softmax rows are scale-invariant, so scale
    row i by exp(-0.2*ai_i):
      p'[j,i] = adj * max(exp(0.8*ai_i)*exp(aj_j), exp(0.2*aj_j))
    pass 1 builds t = exp(0.8*ai)*exp(aj) either as a VectorE
    tensor_scalar_mul against a broadcast of exp(0.8*ai) (rank-1) or as
    a ScalarE Exp activation with per-partition bias aj (2 tiles each
    per quad, balancing the engines); pass 2 is one fused VectorE
    scalar_tensor_tensor: (t max f2) mult adj. Dual-scalar
    tensor_scalar and Pool-engine compute are avoided (measured 9-30x
    slow).
  - No max-subtraction softmax: all fields positive, denominators
    >= 20*exp(-3); bf16 dynamic range covers exp(27).
"""

import numpy as np
import ml_dtypes

B = 2
N = 4096
IN_F = 256
O = 256
NCORES = 8
SLABS_PER_B = 4
NI = N // SLABS_PER_B        # 1024 rows per core
JT = N // 128                # 32 j-tiles
NQ = JT // 4                 # 8 quads of j-tiles
IC = NI // 128               # 8 output chunks per core
KA = IN_F + 1                # 257 (x augmented with ones column)
ALPHA = 0.2

_CACHE = {}


def _build():
    import concourse.bacc as bacc
    import concourse.mybir as mybir
    import concourse.tile as tile

    dt = mybir.dt
    AF = mybir.ActivationFunctionType
    ALU = mybir.AluOpType

    nc = bacc.Bacc("TRN2", target_bir_lowering=False, debug=False,
                   num_devices=NCORES)

    adjsT = nc.dram_tensor("adjsT", [N, NI], dt.bfloat16, kind="ExternalInput")
    x_aug = nc.dram_tensor("x_aug", [128, JT, KA], dt.bfloat16,
                           kind="ExternalInput")
    wT_d = nc.dram_tensor("wT", [128, 2, O], dt.bfloat16, kind="ExternalInput")
    g_row = nc.dram_tensor("g_row", [1, NI], dt.bfloat16, kind="ExternalInput")
    ai08_d = nc.dram_tensor("ai08", [1, NI], dt.float32, kind="ExternalInput")
    f1c_d = nc.dram_tensor("f1c", [128, JT], dt.float32, kind="ExternalInput")
    f2c_d = nc.dram_tensor("f2c", [128, JT], dt.float32, kind="ExternalInput")
    ajc_d = nc.dram_tensor("ajc", [128, JT], dt.float32, kind="ExternalInput")
    ident_d = nc.dram_tensor("ident", [128, 128], dt.bfloat16,
                             kind="ExternalInput")
    out = nc.dram_tensor("out", [NI, O], dt.float32, kind="ExternalOutput")

    adjq_view = adjsT.ap().rearrange("(q k p) i -> q p k i", k=4, p=128)
    adjj_view = adjsT.ap().rearrange("(t p) i -> t p i", p=128)

    with tile.TileContext(nc) as tc:
        with tc.tile_pool(name="consts", bufs=1) as consts, \
             tc.tile_pool(name="adjp", bufs=3) as adjp, \
             tc.tile_pool(name="tvp", bufs=2) as tvp, \
             tc.tile_pool(name="pp", bufs=2) as pp, \
             tc.tile_pool(name="ysbp", bufs=1) as ysbp, \
             tc.tile_pool(name="ytp", bufs=2) as ytp, \
             tc.tile_pool(name="outp", bufs=2) as outp, \
             tc.tile_pool(name="smallp", bufs=2) as smallp:

            # ---- constants; tiny p-build consts ride the gpsimd queue so
            # the Sync queue's first setups are adj jt0 / x q0 ----
            g_bc = consts.tile([128, NI], dt.bfloat16, tag="g_bc")
            nc.gpsimd.dma_start(out=g_bc[:],
                                in_=g_row.ap().to_broadcast([128, NI]))
            f1c = consts.tile([128, JT], dt.float32, tag="f1c")
            f2c = consts.tile([128, JT], dt.float32, tag="f2c")
            ajc = consts.tile([128, JT], dt.float32, tag="ajc")
            nc.gpsimd.dma_start(out=f1c[:], in_=f1c_d[:])
            nc.gpsimd.dma_start(out=f2c[:], in_=f2c_d[:])
            nc.gpsimd.dma_start(out=ajc[:], in_=ajc_d[:])
            ai08_bc = consts.tile([128, NI], dt.float32, tag="ai08_bc")
            nc.gpsimd.dma_start(out=ai08_bc[:],
                                in_=ai08_d.ap().to_broadcast([128, NI]))
            wT_sb = consts.tile([128, 2, O], dt.bfloat16, tag="wT")
            ident = consts.tile([128, 128], dt.bfloat16, tag="ident")
            x_sb = consts.tile([128, JT, KA], dt.bfloat16, tag="x_sb")

            # ---- main loop: Y[ic] += p'.T @ [x|1] over 32 j-tiles ----
            with tc.tile_pool(name="accp", bufs=1, space="PSUM") as accp:
                accs = [accp.tile([128, KA], dt.float32, tag=f"acc{ic}",
                                  name=f"acc{ic}")
                        for ic in range(IC)]
                r_t = smallp.tile([128, IC], dt.float32, tag="r_t")
                ysb = ysbp.tile([128, IC, O], dt.bfloat16, tag="ysb")

                for q in range(NQ):
                    adjt = adjp.tile([128, 4, NI], dt.bfloat16, name="adjt")
                    if q == 0:
                        # jt0 first so the first p-build starts ASAP; x q0
                        # next so the first matmul isn't DMA-setup-gated
                        nc.sync.dma_start(out=adjt[:, 0, :], in_=adjj_view[0])
                        nc.sync.dma_start(out=x_sb[:, 0:4, :],
                                          in_=x_aug[:, 0:4, :])
                        for k in range(1, 4):
                            nc.sync.dma_start(out=adjt[:, k, :],
                                              in_=adjj_view[k])
                        # tail-only consts; issued early, needed late
                        nc.sync.dma_start(out=wT_sb[:], in_=wT_d[:])
                        nc.sync.dma_start(out=ident[:], in_=ident_d[:])
                    else:
                        nc.sync.dma_start(out=adjt[:], in_=adjq_view[q])
                        nc.sync.dma_start(out=x_sb[:, 4 * q:4 * q + 4, :],
                                          in_=x_aug[:, 4 * q:4 * q + 4, :])
                    tv = tvp.tile([128, 4, NI], dt.bfloat16, name="tv")
                    p_t = pp.tile([128, 4, NI], dt.bfloat16, name="p_t")
                    for k in range(4):
                        jt = q * 4 + k
                        js = slice(jt, jt + 1)
                        if k < 2:
                            nc.vector.tensor_scalar_mul(tv[:, k, :], g_bc[:],
                                                        f1c[:, js])
                        else:
                            nc.scalar.activation(tv[:, k, :], ai08_bc[:],
                                                 AF.Exp, bias=ajc[:, js])
                        nc.vector.scalar_tensor_tensor(
                            p_t[:, k, :], tv[:, k, :], f2c[:, js],
                            adjt[:, k, :], op0=ALU.max, op1=ALU.mult)
                    if q < NQ - 1:
                        for k in range(4):
                            jt = q * 4 + k
                            for ic in range(IC):
                                nc.tensor.matmul(
                                    accs[ic][:],
                                    p_t[:, k, ic * 128:(ic + 1) * 128],
                                    x_sb[:, jt, :],
                                    start=(jt == 0), stop=False)
                    else:
                        # last quad ic-major: each acc finishes early so its
                        # reciprocal + normalize-cast pipeline under the
                        # remaining matmuls
                        for ic in range(IC):
                            for k in range(4):
                                jt = q * 4 + k
                                nc.tensor.matmul(
                                    accs[ic][:],
                                    p_t[:, k, ic * 128:(ic + 1) * 128],
                                    x_sb[:, jt, :],
                                    start=False, stop=(k == 3))
                            nc.vector.reciprocal(r_t[:, ic:ic + 1],
                                                 accs[ic][:, O:O + 1])
                            nc.scalar.activation(ysb[:, ic, :],
                                                 accs[ic][:, 0:O],
                                                 AF.Copy,
                                                 scale=r_t[:, ic:ic + 1])

            # ---- tail: out = yhat @ W.T (PE transpose + 16 matmuls);
            # bW is added on the host during unshard ----
            with tc.tile_pool(name="tps", bufs=4, space="PSUM") as tps, \
                 tc.tile_pool(name="g2p", bufs=4, space="PSUM") as g2p:
                for ic in range(IC):
                    yt = ytp.tile([128, 2, 128], dt.bfloat16, name="yt")
                    for fh in range(2):
                        tp = tps.tile([128, 128], dt.bfloat16, name="tp")
                        nc.tensor.transpose(
                            tp[:], ysb[:, ic, fh * 128:(fh + 1) * 128],
                            ident[:])
                        if fh == 0:
                            nc.vector.tensor_copy(yt[:, fh, :], tp[:])
                        else:
                            nc.scalar.copy(yt[:, fh, :], tp[:])
                    G = g2p.tile([128, O], dt.float32, name="G")
                    nc.tensor.matmul(G[:], yt[:, 0, :], wT_sb[:, 0, :],
                                     start=True, stop=False)
                    nc.tensor.matmul(G[:], yt[:, 1, :], wT_sb[:, 1, :],
                                     start=False, stop=True)
                    ot = outp.tile([128, O], dt.float32, name="ot")
                    if ic % 2 == 0:
                        nc.vector.tensor_copy(ot[:], G[:])
                    else:
                        nc.scalar.copy(ot[:], G[:])
                    nc.sync.dma_start(out=out[ic * 128:(ic + 1) * 128, :],
                                      in_=ot[:])

    nc.compile()
    return nc


def _get_nc():
    if "nc" not in _CACHE:
        _CACHE["nc"] = _build()
    return _CACHE["nc"]


def kernel(x, adjs, W, bW, wai, bai, waj, baj):
    from concourse import bass_utils

    bf16 = ml_dtypes.bfloat16
    x = np.asarray(x, np.float32)
    adjs = np.asarray(adjs, np.float32)
    W = np.asarray(W, np.float32)
    bW = np.asarray(bW, np.float32)
    wai = np.asarray(wai, np.float32)
    bai = np.asarray(bai, np.float32)
    waj = np.asarray(waj, np.float32)
    baj = np.asarray(baj, np.float32)

    # host-folded attention projections (f64 for accuracy)
    u_i = W.astype(np.float64).T @ wai.astype(np.float64).T        # [256,1]
    c_i = float(bW.astype(np.float64) @ wai[0].astype(np.float64)
                + bai.astype(np.float64)[0])
    u_j = W.astype(np.float64).T @ waj.astype(np.float64).T
    c_j = float(bW.astype(np.float64) @ waj[0].astype(np.float64)
                + baj.astype(np.float64)[0])
    ai = (x.astype(np.float64) @ u_i)[:, :, 0] + c_i               # [B,N] f64
    aj = (x.astype(np.float64) @ u_j)[:, :, 0] + c_j

    # per-batch shared inputs
    wT = np.ascontiguousarray(
        W.T.reshape(2, 128, O).transpose(1, 0, 2)).astype(bf16)
    ident = np.eye(128, dtype=np.float32).astype(bf16)

    x_aug_b, f1_b, f2_b, aj_b = [], [], [], []
    for b in range(B):
        xa = np.empty((128, JT, KA), bf16)
        xa[:, :, :IN_F] = x[b].reshape(JT, 128, IN_F).transpose(1, 0, 2)
        xa[:, :, IN_F] = np.float32(1.0)
        x_aug_b.append(xa)
        ajr = aj[b].reshape(JT, 128).T
        f1_b.append(np.ascontiguousarray(np.exp(ajr)).astype(np.float32))
        f2_b.append(np.ascontiguousarray(np.exp(ALPHA * ajr)).astype(np.float32))
        aj_b.append(np.ascontiguousarray(ajr).astype(np.float32))

    in_maps = []
    for c in range(NCORES):
        b, s = divmod(c, SLABS_PER_B)
        i0 = s * NI
        adjsT_slab = np.ascontiguousarray(adjs[b][i0:i0 + NI, :].T).astype(bf16)
        ai_slab = ai[b, i0:i0 + NI]
        in_maps.append({
            "adjsT": adjsT_slab,
            "x_aug": x_aug_b[b],
            "wT": wT,
            "g_row": np.exp(0.8 * ai_slab).reshape(1, NI).astype(bf16),
            "ai08": (0.8 * ai_slab).reshape(1, NI).astype(np.float32),
            "f1c": f1_b[b],
            "f2c": f2_b[b],
            "ajc": aj_b[b],
            "ident": ident,
        })

    nc = _get_nc()
    res = bass_utils.run_bass_kernel_spmd(
        nc, in_maps, core_ids=list(range(NCORES)),
        **_CACHE.get("run_kwargs", {}))
    _CACHE["last_results"] = res

    out = np.empty((B, N, O), np.float32)
    for c in range(NCORES):
        b, s = divmod(c, SLABS_PER_B)
        out[b, s * NI:(s + 1) * NI, :] = res.results[c]["out"] + bW
    return out
